# revision 12
# baseline (speedup 1.0000x reference)
"""ClusteredLinear Trainium2 kernel.

out[b, c, p] = sum_s x[b, c, s] * W[clusters[c], p, s] + b[clusters[c], p]

Strategy (8 NeuronCores, SPMD):
  - Batch-parallel: each core gets 8 of the 64 batches.
  - Host sorts channels by cluster id so each cluster's tokens are a
    contiguous run, and packs x/W/out into partition-major, segment-blocked
    DRAM layouts so every DMA moves one large contiguous run per partition
    (~10KB descriptors instead of ~1.7KB).
  - The bias is folded into the matmul via an extra ones-row in x and a bias
    row in W (K=121 per chunk costs the same PE time as K=120).
  - Device computes out^T [336, tokens]: per <=512-token single-cluster
    segment, 3 M-chunks x 6 K-chunks of float32r matmuls accumulate into a
    3-bank PSUM tile; VectorE copies PSUM->SBUF; store on the scalar ring.
  - All 8 clusters' weights stay SBUF-resident, loaded just-in-time.
  - Host undoes the packing on the gathered outputs.

Matmul dtype: float32r (1 cyc/row at N>=256; fp32 is 4 cyc/row and makes the
kernel PE-bound). PSUM accumulation is fp32. Measured absmax-scaled error vs
the fp32 reference: ~1.4e-4 (tf32-class multiplies).
"""

import sys

sys.path.insert(0, "/opt/trn_rl_repo")

import numpy as np

B, C, S, P, NCL = 64, 862, 720, 336, 8
NCORES = 8
BL = B // NCORES          # batches per core
T = C * BL                # tokens per core
KP, NK = 120, 6           # contraction tiles: S = NK * KP
KPA = KP + 1              # +1 ones/bias row per chunk
MP, NM = 112, 3           # output-row tiles: P = NM * MP
MAXL = 512                # max tokens per segment (PSUM bank = 512 f32)
MAX_SEG_CH = MAXL // BL   # max channels per segment

MM_DTYPE = "float32r"
TRACE = False
TRACE_CORES = None

_cache = {}


def _segments(counts):
    """[(cluster, token_start, token_len)] — single-cluster, <=512 tokens."""
    segs = []
    t0 = 0
    for k in range(NCL):
        cnt = int(counts[k])
        if cnt == 0:
            continue
        n = -(-cnt // MAX_SEG_CH)
        base, rem = divmod(cnt, n)
        for j in range(n):
            ch = base + (1 if j < rem else 0)
            segs.append((k, t0, ch * BL))
            t0 += ch * BL
    assert t0 == T
    return segs


def _build(segs, mm_dtype):
    from contextlib import ExitStack

    import concourse.bass as bass
    import concourse.tile as tile
    from concourse import bacc, mybir

    f32 = mybir.dt.float32
    mmdt = getattr(mybir.dt, mm_dtype)

    nc = bacc.Bacc(
        "TRN2",
        target_bir_lowering=False,
        debug=False,
        num_devices=NCORES,
    )
    # Partition-major packed layouts (one contiguous run per partition/DMA):
    #   xt[p, 6*t0 : 6*(t0+L)] = segment's 6 K-chunks back to back
    #   wt[k, p, :]            = cluster k's 6 K-chunks x 336 outputs
    #   out[p, 3*t0 : 3*(t0+L)] = segment's 3 M-chunks back to back
    xt_d = nc.dram_tensor("xt", [KPA, NK * T], mmdt, kind="ExternalInput")
    wt_d = nc.dram_tensor("wt", [NCL, KPA, NK * P], mmdt, kind="ExternalInput")
    out_d = nc.dram_tensor("out", [MP, NM * T], f32, kind="ExternalOutput")

    with tile.TileContext(nc) as tc, ExitStack() as ctx:
        wpool = ctx.enter_context(tc.tile_pool(name="w", bufs=1))
        xpool = ctx.enter_context(tc.tile_pool(name="x", bufs=6))
        opool = ctx.enter_context(tc.tile_pool(name="o", bufs=3))
        pspool = ctx.enter_context(
            tc.tile_pool(name="ps", bufs=2, space=bass.MemorySpace.PSUM)
        )

        # One resident weight tile per cluster, loaded just-in-time so segment
        # k's matmuls only wait for their own cluster's weight DMA, not all 8.
        # Loads/stores alternate between the two HWDGE rings (SP and ACT) so
        # a WAR wait at the head of one ring doesn't serialize the other.
        w_tiles = {}

        def load_w(k, eng):
            w_tiles[k] = wpool.tile(
                [KPA, NK * P], mmdt, tag=f"w{k}", name=f"w{k}"
            )
            eng.dma_start(w_tiles[k][:], wt_d[k])

        for idx, (k, t0, L) in enumerate(segs):
            load_eng = nc.sync if idx % 2 == 0 else nc.scalar
            store_eng = nc.scalar if idx % 2 == 0 else nc.sync
            if k not in w_tiles:
                load_w(k, store_eng)
            x_t = xpool.tile([KPA, NK * L], mmdt, tag="x")
            load_eng.dma_start(x_t[:], xt_d[:, NK * t0 : NK * (t0 + L)])
            ps = pspool.tile([MP, NM, MAXL], f32, tag="ps")
            for m in range(NM):
                for s in range(NK):
                    nc.tensor.matmul(
                        ps[:, m, :L],
                        w_tiles[k][:, s * P + m * MP : s * P + (m + 1) * MP],
                        x_t[:, s * L : (s + 1) * L],
                        start=(s == 0),
                        stop=(s == NK - 1),
                    )
            o_t = opool.tile([MP, NM, L], f32, tag="o")
            nc.vector.tensor_copy(o_t[:], ps[:, :, :L])
            store_eng.dma_start(
                out_d[:, NM * t0 : NM * (t0 + L)],
                o_t[:].rearrange("p m t -> p (m t)"),
            )

    nc.compile()
    return nc


def run(x, clusters, W, b, trace=False, trace_cores=None):
    from concourse.bass_utils import run_bass_kernel_spmd

    x = np.asarray(x, dtype=np.float32)
    clusters = np.asarray(clusters)
    W = np.asarray(W, dtype=np.float32)
    b = np.asarray(b, dtype=np.float32)

    order = np.argsort(clusters, kind="stable")
    counts = np.bincount(clusters.astype(np.int64), minlength=NCL)
    segs = _segments(counts)

    key = (tuple(int(c) for c in counts), MM_DTYPE)
    if key not in _cache:
        _cache[key] = _build(segs, MM_DTYPE)
    nc = _cache[key]

    # Weights: [NCL, KPA, NK*P]; aux row (p=KP) carries bias on chunk 0.
    wt = np.zeros((NCL, KPA, NK * P), dtype=np.float32)
    wt[:, :KP, :] = (
        W.transpose(0, 2, 1).reshape(NCL, NK, KP, P)
        .transpose(0, 2, 1, 3).reshape(NCL, KP, NK * P)
    )
    wt[:, KP, :P] = b

    xs = x[:, order, :]                                      # [64,862,720]
    in_maps = []
    for i in range(NCORES):
        xsT = xs[i * BL : (i + 1) * BL].transpose(2, 1, 0).reshape(S, T)
        xt = np.empty((KPA, NK * T), dtype=np.float32)
        for _, t0, L in segs:
            blk = xsT[:, t0 : t0 + L].reshape(NK, KP, L)
            xt[:KP, NK * t0 : NK * (t0 + L)] = blk.transpose(1, 0, 2).reshape(
                KP, NK * L
            )
            xt[KP, NK * t0 : NK * (t0 + L)] = 1.0
        in_maps.append({"xt": xt, "wt": wt})

    res = run_bass_kernel_spmd(
        nc,
        in_maps,
        list(range(NCORES)),
        trace=trace,
        trace_cores=trace_cores,
    )

    out = np.empty((B, C, P), dtype=np.float32)
    outT = np.empty((NM, MP, T), dtype=np.float32)
    for i in range(NCORES):
        o4 = res.results[i]["out"]                           # [112, 3*T]
        for _, t0, L in segs:
            blk = o4[:, NM * t0 : NM * (t0 + L)].reshape(MP, NM, L)
            outT[:, :, t0 : t0 + L] = blk.transpose(1, 0, 2)
        o = outT.reshape(P, C, BL).transpose(2, 1, 0)        # [BL, C, P]
        out[i * BL : (i + 1) * BL, order, :] = o
    return out, res


def kernel(x, clusters, W, b):
    out, _ = run(x, clusters, W, b, trace=TRACE, trace_cores=TRACE_CORES)
    return out


# revision 24
# speedup vs baseline: 1.4864x; 1.4864x over previous
"""ClusteredLinear Trainium2 kernel.

out[b, c, p] = sum_s x[b, c, s] * W[clusters[c], p, s] + b[clusters[c], p]

Strategy (8 NeuronCores, SPMD):
  - Batch-parallel: each core gets 8 of the 64 batches.
  - Host sorts channels by cluster id so each cluster's tokens are a
    contiguous run, and packs x/W/out into partition-major, segment-blocked
    DRAM layouts so every DMA moves one large contiguous run per partition.
  - DMA partition dim padded to 128 (descriptor count == partition count;
    121 descriptors land on only 11 of 16 SDMA engines, 128 uses all 16).
    Compute slices only the first 121 rows.
  - Segment PAIRS share one x load and one out store (~20KB descriptors to
    amortize the ~230ns/packet SDMA overhead).
  - The bias is folded into the matmul via an extra ones-row in x and a bias
    row in W (K=121 per chunk costs the same PE time as K=120).
  - Device computes out^T [336, tokens]: per <=512-token single-cluster
    segment, 3 M-chunks x 6 K-chunks of float32r matmuls accumulate into a
    3-bank PSUM tile; VectorE copies PSUM->SBUF.
  - All 8 clusters' weights stay SBUF-resident, loaded just-in-time.
  - Loads/stores alternate between the two HWDGE rings (SP and ACT).
  - Host undoes the packing on the gathered outputs.

Matmul dtype: float32r (1 cyc/row at N>=256; fp32 is 4 cyc/row and makes the
kernel PE-bound). PSUM accumulation is fp32. Measured absmax-scaled error vs
the fp32 reference: ~1.4e-4 (tf32-class multiplies).
"""

import sys

sys.path.insert(0, "/opt/trn_rl_repo")

import numpy as np

B, C, S, P, NCL = 64, 862, 720, 336, 8
NCORES = 8
BL = B // NCORES          # batches per core
T = C * BL                # tokens per core
KP, NK = 120, 6           # contraction tiles: S = NK * KP
KPA = KP + 1              # +1 ones/bias row per chunk
KPAD = 128                # DMA partition padding (rows KPA..127 are zeros)
MP, NM = 112, 3           # output-row tiles: P = NM * MP
MAXL = 512                # max tokens per segment (PSUM bank = 512 f32)
MAX_SEG_CH = MAXL // BL   # max channels per segment

MM_DTYPE = "float32r"
TRACE = False
TRACE_CORES = None

_cache = {}


def _segments(counts):
    """[(cluster, token_start, token_len)] — single-cluster, <=512 tokens.

    The very first chunk is kept small (16 channels = 128 tokens) so the
    opening x DMA lands quickly and the matmul pipeline fills early.
    """
    segs = []
    t0 = 0
    ks = [k for k in range(NCL) if counts[k] > 0]
    for k in ks:
        cnt = int(counts[k])
        chunks = []
        if k == ks[0] and cnt > 48:
            # small opening chunk: fast first DMA -> early pipeline fill
            # (32 ch = 256 tokens, the fp32r full-rate minimum)
            chunks.append(32)
            cnt -= 32
        tail = 0
        if k == ks[-1] and cnt > 48:
            # small closing chunk: cheap final copy+store tail
            tail = 32
            cnt -= 32
        n = -(-cnt // MAX_SEG_CH)
        base, rem = divmod(cnt, n)
        chunks += [base + (1 if j < rem else 0) for j in range(n)]
        if tail:
            chunks.append(tail)
        for ch in chunks:
            segs.append((k, t0, ch * BL))
            t0 += ch * BL
    assert t0 == T
    return segs


def _pairs(segs):
    """Group consecutive segments into pairs; the first and last (small)
    segments stay singletons for a fast pipeline fill and a cheap tail."""
    if len(segs) < 4:
        return [tuple(segs[i : i + 2]) for i in range(0, len(segs), 2)]
    groups = [(segs[0],)]
    rest = segs[1:-1]
    groups += [tuple(rest[i : i + 2]) for i in range(0, len(rest), 2)]
    groups.append((segs[-1],))
    return groups


def _build(segs, mm_dtype):
    from contextlib import ExitStack

    import concourse.bass as bass
    import concourse.tile as tile
    from concourse import bacc, mybir

    f32 = mybir.dt.float32
    mmdt = getattr(mybir.dt, mm_dtype)

    nc = bacc.Bacc(
        "TRN2",
        target_bir_lowering=False,
        debug=False,
        num_devices=NCORES,
    )
    # Partition-major packed layouts (one contiguous run per partition/DMA):
    #   xt[p, 6*t0 : 6*(t0+L)] = segment's 6 K-chunks back to back
    #   wt[k, p, :]            = cluster k's 6 K-chunks x 336 outputs
    #   out[p, 3*t0 : 3*(t0+L)] = segment's 3 M-chunks back to back
    xt_d = nc.dram_tensor("xt", [KPAD, NK * T], mmdt, kind="ExternalInput")
    wt_d = nc.dram_tensor("wt", [NCL, KPAD, NK * P], mmdt, kind="ExternalInput")
    out_d = nc.dram_tensor("out", [MP, NM * T], f32, kind="ExternalOutput")

    with tile.TileContext(nc) as tc, ExitStack() as ctx:
        wpool = ctx.enter_context(tc.tile_pool(name="w", bufs=1))
        xpool = ctx.enter_context(tc.tile_pool(name="x", bufs=4))
        opool = ctx.enter_context(tc.tile_pool(name="o", bufs=3))
        pspool = ctx.enter_context(
            tc.tile_pool(name="ps", bufs=6, space=bass.MemorySpace.PSUM)
        )
        warmpool = ctx.enter_context(
            tc.tile_pool(name="warm", bufs=1, space=bass.MemorySpace.PSUM)
        )

        # Pre-warm the PE during the initial DMA window: ~5us of dummy fp32
        # matmuls lift the HAM clock gate to 2.4 GHz before real work lands.
        warm_x = wpool.tile([KPAD, MAXL], f32, tag="warmx")
        nc.gpsimd.memset(warm_x[:], 0.0)
        ps_warm = warmpool.tile([MP, MAXL], f32, tag="warmps")
        for _ in range(8):
            nc.tensor.matmul(
                ps_warm[:],
                warm_x[:KPA, :MP],
                warm_x[:KPA, :],
                start=True,
                stop=True,
            )

        # One resident weight tile per cluster, loaded just-in-time so segment
        # k's matmuls only wait for their own cluster's weight DMA, not all 8.
        w_tiles = {}

        def load_w(k, eng):
            w_tiles[k] = wpool.tile(
                [KPAD, NK * P], mmdt, tag=f"w{k}", name=f"w{k}"
            )
            eng.dma_start(w_tiles[k][:], wt_d[k])

        # Loads own the SP ring; stores + weight loads own the ACT ring, so
        # a store can never head-of-line-block the x stream.
        for idx, pair in enumerate(_pairs(segs)):
            load_eng = nc.sync
            store_eng = nc.scalar
            pt0 = pair[0][1]
            pL = sum(L for _, _, L in pair)
            for k, _, _ in pair:
                if k not in w_tiles:
                    load_w(k, store_eng)
            x_t = xpool.tile([KPAD, NK * pL], mmdt, tag="x")
            load_eng.dma_start(x_t[:], xt_d[:, NK * pt0 : NK * (pt0 + pL)])
            o_t = opool.tile([MP, NM * pL], f32, tag="o")
            for si, (k, t0, L) in enumerate(pair):
                off = t0 - pt0
                for m in range(NM):
                    # Per-m PSUM tile (1 bank) -> finer recycling, and the
                    # per-m copy alternates DVE/ACT so neither engine's copy
                    # chain paces the PE-bound stretch.
                    ps = pspool.tile([MP, MAXL], f32, tag="ps")
                    for s in range(NK):
                        nc.tensor.matmul(
                            ps[:, :L],
                            w_tiles[k][:KPA, s * P + m * MP : s * P + (m + 1) * MP],
                            x_t[:KPA, NK * off + s * L : NK * off + (s + 1) * L],
                            start=(s == 0),
                            stop=(s == NK - 1),
                        )
                    dst = o_t[:, NM * off + m * L : NM * off + (m + 1) * L]
                    if (si + m) % 2 == 0:
                        nc.vector.tensor_copy(dst, ps[:, :L])
                    else:
                        nc.scalar.copy(dst, ps[:, :L])
            store_eng.dma_start(
                out_d[:, NM * pt0 : NM * (pt0 + pL)], o_t[:]
            )

    nc.compile()
    return nc


def run(x, clusters, W, b, trace=False, trace_cores=None):
    from concourse.bass_utils import run_bass_kernel_spmd

    x = np.asarray(x, dtype=np.float32)
    clusters = np.asarray(clusters)
    W = np.asarray(W, dtype=np.float32)
    b = np.asarray(b, dtype=np.float32)

    order = np.argsort(clusters, kind="stable")
    counts = np.bincount(clusters.astype(np.int64), minlength=NCL)
    segs = _segments(counts)

    key = (tuple(int(c) for c in counts), MM_DTYPE)
    if key not in _cache:
        _cache[key] = _build(segs, MM_DTYPE)
    nc = _cache[key]

    # Weights: [NCL, KPAD, NK*P]; aux row (p=KP) carries bias on chunk 0;
    # rows KPA..KPAD-1 are zero DMA ballast.
    wt = np.zeros((NCL, KPAD, NK * P), dtype=np.float32)
    wt[:, :KP, :] = (
        W.transpose(0, 2, 1).reshape(NCL, NK, KP, P)
        .transpose(0, 2, 1, 3).reshape(NCL, KP, NK * P)
    )
    wt[:, KP, :P] = b

    xs = x[:, order, :]                                      # [64,862,720]
    in_maps = []
    for i in range(NCORES):
        xsT = xs[i * BL : (i + 1) * BL].transpose(2, 1, 0).reshape(S, T)
        xt = np.zeros((KPAD, NK * T), dtype=np.float32)
        for _, t0, L in segs:
            blk = xsT[:, t0 : t0 + L].reshape(NK, KP, L)
            xt[:KP, NK * t0 : NK * (t0 + L)] = blk.transpose(1, 0, 2).reshape(
                KP, NK * L
            )
            xt[KP, NK * t0 : NK * (t0 + L)] = 1.0
        in_maps.append({"xt": xt, "wt": wt})

    res = run_bass_kernel_spmd(
        nc,
        in_maps,
        list(range(NCORES)),
        trace=trace,
        trace_cores=trace_cores,
    )

    out = np.empty((B, C, P), dtype=np.float32)
    outT = np.empty((NM, MP, T), dtype=np.float32)
    for i in range(NCORES):
        o4 = res.results[i]["out"]                           # [112, 3*T]
        for _, t0, L in segs:
            blk = o4[:, NM * t0 : NM * (t0 + L)].reshape(MP, NM, L)
            outT[:, :, t0 : t0 + L] = blk.transpose(1, 0, 2)
        o = outT.reshape(P, C, BL).transpose(2, 1, 0)        # [BL, C, P]
        out[i * BL : (i + 1) * BL, order, :] = o
    return out, res


def kernel(x, clusters, W, b):
    out, _ = run(x, clusters, W, b, trace=TRACE, trace_cores=TRACE_CORES)
    return out
